# revision 8
# baseline (speedup 1.0000x reference)
"""CompGCN layer kernel for 8 Trainium2 NeuronCores.

Strategy (dst-sharded, gather + selector-matmul aggregation, no collectives):
  - Each core owns 6250 destination nodes and receives exactly the edges
    whose dst falls in its range (host bucketing).
  - (h[src] + rel[type]) @ W == (h@W)[src] + (rel@W)[type]. Raw h rows are
    gathered, aggregated per dst, then W is applied to the aggregate.
  - Edges are grouped per dst-tile (128 dst nodes) into two exactly-sized
    segments: A (src < 25000) and B (src >= 25000, local idx) — int16 gather
    indices can't span 50048 rows. GPSIMD dma_gather pulls h16[src] for each
    segment (edge-major fp16 tiles). Segment sizes are DATA-DEPENDENT
    (program is built per input): per-tile max-over-cores exact counts,
    idx pads point at row 0 (~6% overhead vs 33% for static sizing).
  - Aggregation is a matmul producing aggT [feat, dst] directly: for each
    128-edge K-tile, the gathered tile st16[e, f] is the stationary operand
    and a one-hot selector S[e, j] = (dst_rel_e == j) (DVE iota + is_equal)
    is the moving operand; PSUM [f, d] accumulates all K-tiles. Pad slots
    carry dst_rel = -1 so their selector column is all-zero.
  - W applied post-aggregation: agg2[d, f'] = matmul(lhsT=aggT_sbuf, rhs=W);
    rel contribution folds in as ct^T @ relW (ct = per-core dst x type count
    histogram, host-built, fp16-exact) accumulated into the same PSUM.
  - Fused final: out = relu(agg2 * norm + h_slice @ loop_weight) per tile.
  - dma_gather HW limit: <= 1024 idxs per op (SWDGE desc ring).
"""

import os
import numpy as np
from dataclasses import dataclass
from contextlib import ExitStack

from concourse import bacc, bass, mybir, tile
from concourse.bass_utils import run_bass_kernel_spmd

F32 = mybir.dt.float32
F16 = mybir.dt.float16
I16 = mybir.dt.int16

GCHUNK = int(os.environ.get("KERNEL_GCHUNK", "1024"))  # idxs per dma_gather


@dataclass(frozen=True)
class Cfg:
    n_nodes: int = 50000
    d: int = 128
    n_rels: int = 500
    n_cores: int = 8
    split: int = 25000
    n_edges: int = 600000

    @property
    def np_core(self):
        return self.n_nodes // self.n_cores

    @property
    def np_pad(self):
        return ((self.np_core + 1 + 127) // 128) * 128

    @property
    def n_pad(self):
        return ((self.n_nodes + 127) // 128) * 128

    @property
    def r_pad(self):
        return ((self.n_rels + 127) // 128) * 128


CFG = Cfg()


@dataclass(frozen=True)
class SegPlan:
    """Per-core data-dependent segment plan (all cores share one program).

    kA/kB: K-tiles (128-edge groups) per dst-tile for stream A/B.
    nidxA/nidxB: gather idx counts (multiple of 16, -1 padded) per dst-tile.
    """
    kA: tuple
    kB: tuple
    nidxA: tuple
    nidxB: tuple

    @property
    def n_tiles(self):
        return len(self.kA)

    @property
    def k_tot(self):          # selector/dst_rel K-tile count, all tiles
        return sum(self.kA) + sum(self.kB)

    @property
    def k_max(self):
        return max(ka + kb for ka, kb in zip(self.kA, self.kB))

    @property
    def idx16_tot(self):      # src idx stream length in 16-granules
        return (sum(self.nidxA) + sum(self.nidxB)) // 16


def build_program(cfg: Cfg, plan: SegPlan):
    nc = bacc.Bacc("TRN2", target_bir_lowering=False, debug=False,
                   num_swdge_queues=4)
    D = cfg.d
    n_slice_tiles = cfg.np_pad // 128
    n_r_tiles = cfg.r_pad // 128
    assert plan.n_tiles == n_slice_tiles
    k_max = plan.k_max

    h16 = nc.dram_tensor("h16", [cfg.n_pad, D], F16, kind="ExternalInput")
    rel_t = nc.dram_tensor("rel_t", [D, cfg.r_pad], F16, kind="ExternalInput")
    w_nb = nc.dram_tensor("w_nb", [D, D], F32, kind="ExternalInput")
    w_loop = nc.dram_tensor("w_loop", [D, D], F32, kind="ExternalInput")
    hs_t = nc.dram_tensor("hs_t", [D, cfg.np_pad], F16, kind="ExternalInput")
    norm_row = nc.dram_tensor("norm_row", [D, cfg.np_pad], F32, kind="ExternalInput")
    ct_pack = nc.dram_tensor("ct_pack", [n_slice_tiles, 128, n_r_tiles, 128], F16, kind="ExternalInput")
    src_idx = nc.dram_tensor("src_idx", [128, plan.idx16_tot], I16, kind="ExternalInput")
    dst_rel = nc.dram_tensor("dst_rel", [128, plan.k_tot], F16, kind="ExternalInput")

    out = nc.dram_tensor("out", [cfg.np_pad, D], F32, kind="ExternalOutput")

    with tile.TileContext(nc) as tc:
        with ExitStack() as ex:
            cpool = ex.enter_context(tc.tile_pool(name="const", bufs=1))
            inpool = ex.enter_context(tc.tile_pool(name="ld", bufs=4))
            ctpool = ex.enter_context(tc.tile_pool(name="ct", bufs=4))
            pwpool = ex.enter_context(tc.tile_pool(name="psW", bufs=2, space="PSUM"))
            agpool = ex.enter_context(tc.tile_pool(name="psA", bufs=2, space="PSUM"))
            a2pool = ex.enter_context(tc.tile_pool(name="psB", bufs=2, space="PSUM"))
            segpool = ex.enter_context(tc.tile_pool(name="seg", bufs=6))
            selpool = ex.enter_context(tc.tile_pool(name="sel", bufs=6))
            fpool = ex.enter_context(tc.tile_pool(name="fin", bufs=4))

            # ------- constants (src/dst streams first: they gate the gathers;
            # bulk constants go on the scalar HWDGE queue in parallel) -------
            src_sb = cpool.tile([128, plan.idx16_tot], I16)
            nc.sync.dma_start(out=src_sb[:], in_=src_idx[:, :])
            dc_sb = cpool.tile([128, plan.k_tot], F16)
            nc.sync.dma_start(out=dc_sb[:], in_=dst_rel[:, :])
            iota_big = cpool.tile([128, k_max, 128], F16)
            nc.gpsimd.iota(iota_big[:], pattern=[[0, k_max], [1, 128]], base=0,
                           channel_multiplier=0,
                           allow_small_or_imprecise_dtypes=True)
            w_sb = cpool.tile([128, D], F32)
            nc.scalar.dma_start(out=w_sb[:], in_=w_nb[:, :])
            wl_sb = cpool.tile([128, D], F32)
            nc.scalar.dma_start(out=wl_sb[:], in_=w_loop[:, :])
            w16 = cpool.tile([128, D], F16)
            nc.vector.tensor_copy(out=w16[:], in_=w_sb[:])
            wl16 = cpool.tile([128, D], F16)
            nc.vector.tensor_copy(out=wl16[:], in_=wl_sb[:])
            nr_sb = cpool.tile([128, cfg.np_pad], F32)
            nc.scalar.dma_start(out=nr_sb[:], in_=norm_row[:, :])
            hs_sb = cpool.tile([128, cfg.np_pad], F16)
            nc.scalar.dma_start(out=hs_sb[:], in_=hs_t[:, :])

            relw16 = cpool.tile([128, n_r_tiles, D], F16)
            for rt in range(n_r_tiles):
                rtile = inpool.tile([128, D], F16, tag="rt16")
                nc.scalar.dma_start(out=rtile[:], in_=rel_t[:, rt * D:(rt + 1) * D])
                pw = pwpool.tile([128, D], F32, tag="pw")
                nc.tensor.matmul(out=pw[:], lhsT=rtile[:], rhs=w16[:],
                                 start=True, stop=True)
                nc.vector.tensor_copy(out=relw16[:, rt, :], in_=pw[:])

            # pre-zero the 6 segment buffers (first-use NaN safety: pad/stale
            # slots are multiplied by an all-zero selector column; 0*finite=0
            # but 0*NaN would poison the PSUM)
            seg_bufs = []
            for i in range(6):
                st = segpool.tile([128, k_max, D], F16, tag="seg")
                nc.vector.memset(st[:], 0.0)
                seg_bufs.append(st)

            # ------- per dst-tile: gather + selector matmuls + fused finish -------
            qctr = [0]

            def nextq():
                qctr[0] += 1
                return qctr[0] % 4

            o16 = 0      # running src idx stream offset, 16-granules
            kofs = 0     # running dst_rel / selector K-tile offset
            for t in range(n_slice_tiles):
                kA, kB = plan.kA[t], plan.kB[t]
                nA, nB = plan.nidxA[t], plan.nidxB[t]
                kt = kA + kB
                st16 = seg_bufs[t % 6]
                for part_k0, nidx, tab in ((0, nA, h16[0:cfg.split, :]),
                                           (kA, nB, h16[cfg.split:cfg.n_pad, :])):
                    done = 0
                    while done < nidx:
                        n = min(GCHUNK, nidx - done)
                        ko = part_k0 + done // 128
                        kn = (done + n + 127) // 128 - done // 128
                        nc.gpsimd.dma_gather(
                            out_ap=st16[:, ko:ko + kn, :],
                            in_ap=tab, idxs_ap=src_sb[:, o16:o16 + n // 16],
                            num_idxs=n, num_idxs_reg=n, elem_size=D,
                            queue_num=nextq())
                        o16 += n // 16
                        done += n

                # selector build: one-hot over dst-in-tile for each K-tile
                sel = selpool.tile([128, k_max, 128], F16, tag="sel")
                dc_col = dc_sb[:, kofs:kofs + kt]
                nc.vector.tensor_tensor(
                    out=sel[:, :kt, :], in0=iota_big[:, :kt, :],
                    in1=dc_col.rearrange("p (c o) -> p c o", o=1).broadcast_to(
                        [128, kt, 128]),
                    op=mybir.AluOpType.is_equal)
                kofs += kt

                # aggT[f, d] = sum_e st16[e, f] * sel[e, d]
                aggT = agpool.tile([128, 128], F32, tag="aggT")
                for k in range(kt):
                    nc.tensor.matmul(out=aggT[:], lhsT=st16[:, k, :],
                                     rhs=sel[:, k, :],
                                     start=(k == 0), stop=(k == kt - 1))
                # fold the per-dst norm while moving PSUM->SBUF. tensor_tensor
                # is 2x_1P-only on DVE, so it can't block SWDGE descgen (copy/
                # tensor_scalar 2-port modes would).
                agrt = fpool.tile([128, 128], F32, tag="agrt")
                nc.vector.tensor_tensor(
                    out=agrt[:], in0=aggT[:],
                    in1=nr_sb[:, t * 128:(t + 1) * 128],
                    op=mybir.AluOpType.mult)

                # agg2[d, f'] = norm*(aggT^T @ W + ct^T @ relW) + h_slice @ Wl
                # (ct is pre-scaled by norm on the host; loop message joins the
                # same PSUM group so the finish is one Relu on the ACT engine)
                agg2 = a2pool.tile([128, D], F32, tag="agg2")
                nc.tensor.matmul(out=agg2[:], lhsT=agrt[:], rhs=w_sb[:],
                                 start=True, stop=False)
                ct = ctpool.tile([128, n_r_tiles, 128], F16, tag="ct_ld")
                nc.scalar.dma_start(out=ct[:], in_=ct_pack[t, :, :, :])
                for rt in range(n_r_tiles):
                    nc.tensor.matmul(out=agg2[:], lhsT=ct[:, rt, :], rhs=relw16[:, rt, :],
                                     start=False, stop=False)
                nc.tensor.matmul(out=agg2[:], lhsT=hs_sb[:, t * 128:(t + 1) * 128],
                                 rhs=wl16[:], start=False, stop=True)

                m = fpool.tile([128, D], F32)
                nc.scalar.activation(out=m[:], in_=agg2[:],
                                     func=mybir.ActivationFunctionType.Relu)
                nc.sync.dma_start(out=out[t * 128:(t + 1) * 128, :], in_=m[:])

    nc.compile()
    return nc


def _wrap16(vals: np.ndarray) -> np.ndarray:
    """int16 idx stream [len] -> [128, len//16] (16-wrap, replicated x8)."""
    w16 = vals.astype(np.int16).reshape(-1, 16).T
    return np.tile(w16, (8, 1)).copy()


def prep_inputs(cfg: Cfg, h, norm, rel_emb, w_nb, w_loop, edge_src, edge_dst, edge_type):
    h = np.asarray(h, np.float32)
    norm = np.asarray(norm, np.float32).reshape(-1)
    rel_emb = np.asarray(rel_emb, np.float32)
    edge_src = np.asarray(edge_src, np.int64)
    edge_dst = np.asarray(edge_dst, np.int64)
    edge_type = np.asarray(edge_type, np.int64)

    n_slice_tiles = cfg.np_pad // 128

    h_pad = np.zeros((cfg.n_pad, cfg.d), np.float32)
    h_pad[:cfg.n_nodes] = h
    r_pad = np.zeros((cfg.r_pad, cfg.d), np.float32)
    r_pad[:cfg.n_rels] = rel_emb
    h16 = np.ascontiguousarray(h_pad.astype(np.float16))
    rel_t = np.ascontiguousarray(r_pad.T.astype(np.float16))

    # ---- per-core bucketing + exact per-(tile, part) sizing ----
    per_core = []
    kA_all = np.zeros((cfg.n_cores, n_slice_tiles), np.int64)
    kB_all = np.zeros((cfg.n_cores, n_slice_tiles), np.int64)
    nidxA_all = np.zeros((cfg.n_cores, n_slice_tiles), np.int64)
    nidxB_all = np.zeros((cfg.n_cores, n_slice_tiles), np.int64)
    for c in range(cfg.n_cores):
        lo, hi = c * cfg.np_core, (c + 1) * cfg.np_core
        sel = (edge_dst >= lo) & (edge_dst < hi)
        src_c, dst_c, typ_c = edge_src[sel], edge_dst[sel] - lo, edge_type[sel]
        dtile = dst_c // 128
        in_a = src_c < cfg.split
        parts = []
        for t in range(n_slice_tiles):
            tm = dtile == t
            sA = np.sort(src_c[tm & in_a])
            dA = dst_c[tm & in_a][np.argsort(src_c[tm & in_a], kind="stable")] - t * 128
            sB = np.sort(src_c[tm & ~in_a]) - cfg.split
            dB = dst_c[tm & ~in_a][np.argsort(src_c[tm & ~in_a], kind="stable")] - t * 128
            parts.append((sA, dA, sB, dB))
            kA_all[c, t] = (len(sA) + 127) // 128
            kB_all[c, t] = (len(sB) + 127) // 128
            nidxA_all[c, t] = (len(sA) + 15) // 16 * 16
            nidxB_all[c, t] = (len(sB) + 15) // 16 * 16
        per_core.append((parts, src_c, dst_c, typ_c, lo, hi))

    # all cores share one program: take the per-tile MAX over cores so the
    # layout covers every core (cost: a few pad K-tiles / idx granules)
    kA = tuple(int(x) for x in kA_all.max(axis=0))
    kB = tuple(int(x) for x in kB_all.max(axis=0))
    nidxA = tuple(int(x) for x in nidxA_all.max(axis=0))
    nidxB = tuple(int(x) for x in nidxB_all.max(axis=0))
    # nidx must fit inside the layout region (it does: ceil16(n) <= 128*ceil128(n))
    plan = SegPlan(kA=kA, kB=kB, nidxA=nidxA, nidxB=nidxB)

    in_maps = []
    for c in range(cfg.n_cores):
        parts, src_c, dst_c, typ_c, lo, hi = per_core[c]
        # pads point at row 0 (valid; their selector column is all-zero).
        # -1 trim can't be used: num_idxs_reg is baked into the shared
        # program but real counts differ per core.
        src_stream = np.zeros(plan.idx16_tot * 16, np.int64)
        dc_stream = np.full(plan.k_tot * 128, -1.0, np.float32)
        o = 0      # idx stream offset (elements)
        ko = 0     # dst_rel K-tile offset
        for t in range(n_slice_tiles):
            sA, dA, sB, dB = parts[t]
            for (s, d, nidx, k) in ((sA, dA, plan.nidxA[t], plan.kA[t]),
                                    (sB, dB, plan.nidxB[t], plan.kB[t])):
                # idx stream: real idxs then -1 pads to nidx (trimmed by Q7)
                src_stream[o:o + len(s)] = s
                o += nidx
                # dst_rel: real dsts at the same within-part positions
                dc_stream[ko * 128:ko * 128 + len(d)] = d
                ko += k

        hsl = np.zeros((cfg.np_pad, cfg.d), np.float32)
        hsl[:cfg.np_core] = h[lo:hi]
        hsl_t = np.ascontiguousarray(hsl.T.astype(np.float16))
        ntmp = np.zeros(cfg.np_pad, np.float32)
        ntmp[:cfg.np_core] = norm[lo:hi]
        nrow = np.ascontiguousarray(np.tile(ntmp[None, :], (cfg.d, 1)))

        # ct pre-scaled by the per-dst norm (counts * norm stays fp16-clean)
        ct = np.zeros((cfg.r_pad, cfg.np_pad), np.float32)
        np.add.at(ct, (typ_c, dst_c), 1.0)
        ct *= ntmp[None, :]
        ctp = np.ascontiguousarray(
            ct.reshape(cfg.r_pad // 128, 128, n_slice_tiles, 128)
            .transpose(2, 1, 0, 3)).astype(np.float16)

        in_maps.append({
            "h16": h16, "rel_t": rel_t,
            "w_nb": np.asarray(w_nb, np.float32), "w_loop": np.asarray(w_loop, np.float32),
            "hs_t": hsl_t, "norm_row": nrow,
            "ct_pack": ctp,
            "src_idx": _wrap16(src_stream),
            "dst_rel": np.ascontiguousarray(
                dc_stream.reshape(plan.k_tot, 128).T).astype(np.float16),
        })
    return plan, in_maps


LAST_RESULTS = None


def kernel(h, norm, rel_emb, W_neighbor, loop_weight, edge_src, edge_dst, edge_type):
    cfg = CFG
    plan, in_maps = prep_inputs(cfg, h, norm, rel_emb, W_neighbor, loop_weight,
                                edge_src, edge_dst, edge_type)
    nc = build_program(cfg, plan)
    trace = os.environ.get("KERNEL_TRACE", "0") == "1"
    res = run_bass_kernel_spmd(nc, in_maps, list(range(cfg.n_cores)), trace=trace)
    global LAST_RESULTS
    LAST_RESULTS = res
    outs = [res.results[c]["out"][:cfg.np_core] for c in range(cfg.n_cores)]
    return np.concatenate(outs, axis=0).astype(np.float32)
